# revision 28
# baseline (speedup 1.0000x reference)
"""Distributed causal self-attention with RoPE for 8 TRN2 NeuronCores.

Sharding (Megatron-style, per the hint): head-parallel. Core c owns heads
(2c, 2c+1) for both batch elements. c_attn is column-parallel (each core
computes q/k/v only for its heads from the full x), attention is fully local
per head, and c_proj is row-parallel (each core multiplies its 128 head
channels into a full-width partial output). The 8 partial outputs are summed
on the host during unsharding — no on-device collective is needed.

Per-core kernel layout choices:
  - x is passed pre-transposed as xT [C, B*T] (bf16): QKV runs as
    qT = Wq^T @ xT giving q^T in [head_dim, t] layout, which is exactly the
    lhsT/rhs layout the attention matmuls want (contraction over d).
  - xT streams in as five >=1MB column-range DMAs (k-tiles batched via a
    3D access pattern) on the two HWDGE queues; tables/weights go on the
    SWDGE queue. The previous per-(k,chunk) schedule left the PE starved
    and HAM-cold for the first 45us.
  - The PE is kept HAM-warm through the input-bound start with a stream of
    dummy N=256 matmuls on a zeroed tile (free: HAM only counts busy-ness).
  - RoPE pairs are interleaved on adjacent partitions (host permutes the
    q/k weight columns and the cos/sin tables identically; scores are
    invariant), so the half-rotation is a swap of adjacent partitions done
    by one DVE stream_shuffle instead of four SBUF->SBUF DMAs.
  - v is computed in [t, d] layout (lhsT = xT tile), augmented with a
    ones-column so the PV matmul yT = v_aug^T @ exp(S^T) yields the softmax
    denominator in its last row for free.
  - Softmax skips the running-max subtraction: scores are ~N(0,1) after the
    1/sqrt(d) scale, so exp never overflows fp32; exp runs on the scalar
    engine straight out of PSUM, writing bf16.
  - Causality is exploited at tile granularity (strictly-upper tiles are
    skipped; diagonal tiles stream partial columns and get a triangular
    mask multiply after exp, both heads in one 3D-AP op).
  - Normalization: ln(den) is taken per head row ([1,512], fp16), broadcast
    onto the partition blocks with two accumulating K=1 matmuls, and a
    single [128,512] Exp(scale=-1) produces 1/den in SBUF; the numerators
    are multiplied straight out of the yT PSUM banks (no staging copy).
  - c_proj runs transposed (out^T = Wo^T @ yT); its bias (plus the folded
    v-bias) is added once on the host after the partial sums.
  - Emission order is software-pipelined for Tile's static engine queues:
    PV lags S by several iterations, each block's normalize/proj matmuls
    are deferred into the next block's stream, and the next group's QKV
    matmuls are spliced in as feeders so the tensor engine never idles.
"""

import os
import sys
import types

import numpy as np
import ml_dtypes

import concourse.bass as bass
import concourse.mybir as mybir
from concourse.tile import TileContext
from concourse.vector_clock import ScopedClock

BF16 = mybir.dt.bfloat16
F16 = mybir.dt.float16
F32 = mybir.dt.float32

N_CORES = 8
B, T, C = 2, 2048, 1024
H, D = 16, 64
HPC = H // N_CORES  # heads per core
HD = HPC * D  # local head width = 128
TT = B * T  # flattened tokens = 4096
NK = C // 128  # contraction tiles for QKV
NBLK = T // 512  # tq blocks per batch
NTK = T // 128  # tk tiles per batch
SCALE = float(D) ** -0.5
ROPE_THETA = 10000.0

# swap adjacent partitions within each 32-lane quadrant (rope half-rotation
# in the pair-interleaved channel layout)
SWAP_MASK = [j ^ 1 for j in range(32)]


def _install_axon_hooks_shim():
    """Best-effort: some environments lack antenv.axon_hooks, which
    run_bass_kernel_spmd imports when BASS_TRACE is set. Provide a minimal
    implementation backed by the slim trn boot module if available."""
    try:
        import antenv.axon_hooks  # noqa: F401

        return
    except ImportError:
        pass
    try:
        hook = [None]
        mod = types.ModuleType("antenv.axon_hooks")
        mod.set_axon_ntff_profile_hook = lambda h: hook.__setitem__(0, h)
        mod.get_axon_ntff_profile_hook = lambda: hook[0]
        try:
            from trn_agent_boot.trn_boot import _ntff_profile_via_ctypes

            so = "/opt/axon/libaxon_pjrt.so"
            if os.path.exists(so):
                hook[0] = _ntff_profile_via_ctypes(so)
        except Exception:
            pass
        sys.modules["antenv.axon_hooks"] = mod
        import antenv

        antenv.axon_hooks = mod
    except Exception:
        pass


_install_axon_hooks_shim()


class _TileContextSplitDrain(TileContext):
    """This walrus build rejects >2 sync-waits on one instruction; the Tile
    kernel-tail drain can carry more. Split them across single-wait NOPs."""

    def _drain_and_barrier(self, tick_clock, wait_clock):
        drain_inst = self.nc.sync.drain()
        wait_clock.add_sem_waits(
            drain_inst.ins, ScopedClock({None: tick_clock.global_clock})
        )
        waits = list(drain_inst.ins.sync_info.on_wait)
        if len(waits) > 1:
            drain_inst.ins.sync_info.on_wait[:] = waits[:1]
            for w in waits[1:]:
                nop = self.nc.sync.nop(nofuse=True)
                nop.ins.sync_info = mybir.SyncInfo(on_wait=[w], on_update=[])

        self.nc.all_engine_barrier()
        assert self.sems is not None
        popped = self.nc._tile_sem_poison_stack.pop()
        assert popped is self._sem_poison
        self.nc.clear_and_free_semaphores(list(self.sems.allocated().values()))
        self.nc.all_engine_barrier()


def _split_excess_waits(nc: bass.Bass, limit: int = 1) -> int:
    """This walrus build encodes only a small number of sync-waits per
    instruction; Tile's semaphore assignment can attach more. Hoist excess
    waits onto same-engine NOPs placed immediately before the instruction —
    semantically identical since engine queues execute in order."""
    import bass_rust

    ctr = 0
    for fn in nc.m.functions:
        for bb in fn.blocks:
            insts = bb.instructions
            new = []
            for inst in insts:
                si = inst.sync_info
                waits = list(si.on_wait) if si is not None else []
                if len(waits) > limit:
                    keep = waits[-limit:]
                    extra = waits[: -limit]
                    for s in range(0, len(extra), limit):
                        chunk = extra[s : s + limit]
                        ctr += 1
                        nop = bass_rust.InstNoOp(
                            name=f"I-wsplit{ctr}",
                            engine=inst.engine,
                            ins=[],
                            outs=[],
                            sync_info=mybir.SyncInfo(
                                on_wait=chunk, on_update=[]
                            ),
                        )
                        nc.register_instruction(nop)
                        new.append(nop)
                    si.on_wait[:] = keep
                new.append(inst)
            insts[:] = new
    return ctr


def _build_nc() -> bass.Bass:
    nc = bass.Bass()

    # x arrives host-blocked as xTb[p, g*NK*512 + k*512 + c] = x[k*128+p,
    # g*512+c]: every column-group load is then a plain contiguous 2D slice
    # with 8KB per-partition lines (a [p, k, c] 3D access pattern emits 1KB
    # descriptors and runs at ~100-200 GB/s; contiguous hits ~340).
    xT = nc.declare_dram_parameter("xT", [128, NK * TT], BF16, isOutput=False)
    wq = nc.declare_dram_parameter("wq", [128, C], BF16, isOutput=False)
    wk = nc.declare_dram_parameter("wk", [128, C], BF16, isOutput=False)
    wv = nc.declare_dram_parameter("wv", [128, C], BF16, isOutput=False)
    wo = nc.declare_dram_parameter("wo", [HD, C], BF16, isOutput=False)
    # small constants fused into few tensors: every extra DMA costs ~2us of
    # FIFO completion latency on its queue during the critical startup.
    bqkd = nc.declare_dram_parameter("bqk", [128, 2], F32, isOutput=False)
    csd = nc.declare_dram_parameter("cs", [128, 2 * T], BF16, isOutput=False)
    etd = nc.declare_dram_parameter("eyetri", [128, 384], BF16, isOutput=False)
    outd = nc.declare_dram_parameter("out", [C, TT], BF16, isOutput=True)

    Exp = mybir.ActivationFunctionType.Exp
    Ln = mybir.ActivationFunctionType.Ln

    with _TileContextSplitDrain(nc) as tc:
        with (
            tc.tile_pool(name="consts", bufs=1) as cp,
            tc.tile_pool(name="xt", bufs=1) as xtp,
            tc.tile_pool(name="qk", bufs=1) as qkp,
            tc.tile_pool(name="qsw", bufs=4) as qswp,
            tc.tile_pool(name="rot", bufs=1) as rotp,
            tc.tile_pool(name="vaug", bufs=B * NBLK) as vaugp,
            tc.tile_pool(name="apool", bufs=6) as apool,
            tc.tile_pool(name="yb", bufs=1) as ybp,
            tc.tile_pool(name="rsmall", bufs=2) as rsp,
            tc.tile_pool(name="rinv", bufs=2) as rip,
            tc.tile_pool(name="osb", bufs=6) as osbp,
            tc.tile_pool(name="psmix", bufs=2, space="PSUM") as psmix,
            tc.tile_pool(name="pss", bufs=2, space="PSUM") as pss,
            tc.tile_pool(name="psy0", bufs=1, space="PSUM") as psy0,
            tc.tile_pool(name="psy1", bufs=1, space="PSUM") as psy1,
        ):
            # ---- constants / weights -------------------------------------
            wq_t = cp.tile([128, C], BF16, tag="wq")
            wk_t = cp.tile([128, C], BF16, tag="wk")
            wv_t = cp.tile([128, C], BF16, tag="wv")
            wo_t = cp.tile([HD, C], BF16, tag="wo")
            bqk_t = cp.tile([128, 2], F32, tag="bqk")
            cs_t = cp.tile([128, 2 * T], BF16, tag="cs")
            et_t = cp.tile([128, 384], BF16, tag="eyetri")
            e2a_t = cp.tile([1, 128], F16, tag="e2a")
            e2b_t = cp.tile([1, 128], F16, tag="e2b")
            warm_t = cp.tile([128, 256], BF16, tag="warm")
            bq_t = bqk_t[:, 0:1]
            bk_t = bqk_t[:, 1:2]
            eye_t = et_t[:, 0:128]
            tri_t = et_t[:, 128:384]

            xt_all = xtp.tile([128, NK * TT], BF16, tag="xt")
            cs3 = cs_t[:, :].rearrange("p (s c) -> p s c", s=2)
            cs3d = csd[:, :].rearrange("p (s c) -> p s c", s=2)

            def xt_rhs(g, k):
                o = (g * NK + k) * 512
                return xt_all[:, o : o + 512]

            # Input DMAs. Each queue executes its DMAs in FIFO order with
            # ~2us completion latency apiece, so the three queues carry
            # independent critical chains. Group 0 of xT is split across the
            # two HWDGE queues so the first QKV chunk unblocks earliest; the
            # rest streams on sync in 1-2MB contiguous pieces.
            GRP = NK * 512  # 4096 free elems per column-group
            nc.sync.dma_start(
                out=xt_all[:, 0 : GRP // 2], in_=xT[:, 0 : GRP // 2]
            )
            nc.scalar.dma_start(
                out=xt_all[:, GRP // 2 : GRP], in_=xT[:, GRP // 2 : GRP]
            )
            nc.sync.dma_start(
                out=xt_all[:, GRP : 2 * GRP], in_=xT[:, GRP : 2 * GRP]
            )
            nc.sync.dma_start(
                out=xt_all[:, 2 * GRP : 4 * GRP], in_=xT[:, 2 * GRP : 4 * GRP]
            )
            nc.sync.dma_start(
                out=xt_all[:, 4 * GRP : 6 * GRP], in_=xT[:, 4 * GRP : 6 * GRP]
            )
            nc.sync.dma_start(
                out=xt_all[:, 6 * GRP : 8 * GRP], in_=xT[:, 6 * GRP : 8 * GRP]
            )
            nc.gpsimd.dma_start(out=wq_t[:, :], in_=wq[:, :])
            nc.scalar.dma_start(out=wk_t[:, :], in_=wk[:, :])
            nc.scalar.dma_start(out=wv_t[:, :], in_=wv[:, :])
            nc.scalar.dma_start(out=wo_t[:, :], in_=wo[:, :])
            nc.gpsimd.dma_start(out=bqk_t[:, :], in_=bqkd[:, :])
            nc.gpsimd.dma_start(out=cs3[:, :, 0:512], in_=cs3d[:, :, 0:512])
            nc.gpsimd.dma_start(out=et_t[:, :], in_=etd[:, :])
            nc.gpsimd.dma_start(
                out=cs3[:, :, 512:1024], in_=cs3d[:, :, 512:1024]
            )
            nc.gpsimd.dma_start(
                out=cs3[:, :, 1024:1536], in_=cs3d[:, :, 1024:1536]
            )
            nc.gpsimd.dma_start(
                out=cs3[:, :, 1536:2048], in_=cs3d[:, :, 1536:2048]
            )

            # e2a/e2b: indicator rows of each head's 64-partition block; two
            # accumulating K=1 matmuls broadcast each head's ln(den) row onto
            # its partition block of one [128, 512] PSUM tile.
            nc.vector.memset(e2a_t[:, :], 0.0)
            nc.vector.memset(e2b_t[:, :], 0.0)
            nc.vector.memset(e2a_t[0:1, 0:64], 1.0)
            nc.vector.memset(e2b_t[0:1, 64:128], 1.0)
            nc.vector.memset(warm_t[:, :], 0.0)

            # HAM warmup: the PE's clock gate only opens after ~3.4us of
            # sustained busy-ness, and the input stream keeps real matmuls
            # sparse for the first ~8us. Dummy matmuls on the zeroed tile
            # are free work that trips the gate and bridges DMA jitter.
            # They write the psy banks (idle until block 0's first PV, and
            # same-engine FIFO order means no stall is possible).
            dummy_i = [0]

            def dummy_mms(n, cols=256):
                for _ in range(n):
                    pool, tg = ((psy0, "yt0"), (psy1, "yt1"))[dummy_i[0] % 2]
                    dummy_i[0] += 1
                    pd = pool.tile([128, 256], F32, tag=tg)
                    nc.tensor.matmul(
                        pd[:, 0:cols], warm_t[:, 0:128], warm_t[:, 0:cols],
                        start=True, stop=True,
                    )

            # keep-warm filler usable mid-kernel: every PSUM bank is in an
            # active pool rotation (a pool-allocated dummy would inherit a
            # WAR wait and stall the PE FIFO on the very chain it should
            # hide). Instead write the unused partitions 96:128 of the live
            # yt banks with start=False — per-element has_written means the
            # open PV accumulation on partitions 0:65 is untouched.
            cur_yt = [None, None]

            def dummy_yt(n, cols=128):
                for _ in range(n):
                    ytd = cur_yt[dummy_i[0] % 2]
                    dummy_i[0] += 1
                    nc.tensor.matmul(
                        ytd[96:128, 0:cols], warm_t[:, 0:32],
                        warm_t[:, 0:cols], start=False, stop=False,
                        skip_group_check=True, tile_position=(0, 96),
                    )

            q_sb = qkp.tile([128, TT], BF16, tag="q_sb")
            k_sb = qkp.tile([128, TT], BF16, tag="k_sb")
            vt_sb = qkp.tile([128, TT], BF16, tag="vt_sb")
            qr = rotp.tile([128, TT], BF16, tag="qr")
            kr = rotp.tile([128, TT], BF16, tag="kr")
            yb = ybp.tile([HD, TT], BF16, tag="yb")
            vaugs = [None] * (B * NBLK)

            def qkv_one(g, dst, w_t, b_t, warm=0):
                cols = slice(g * 512, (g + 1) * 512)
                ps = psmix.tile([128, 512], F32, tag="mix")
                for k in range(NK):
                    nc.tensor.matmul(
                        ps[:, :],
                        w_t[:, k * 128 : (k + 1) * 128],
                        xt_rhs(g, k),
                        start=(k == 0),
                        stop=(k == NK - 1),
                    )
                    if warm and k % 2 == 1:
                        dummy_mms(warm)
                if b_t is None:
                    nc.vector.tensor_copy(dst[:, cols], ps[:, :])
                else:
                    nc.vector.tensor_scalar_add(dst[:, cols], ps[:, :], b_t[:, 0:1])

            def qkv_chunk(g, warm=0):
                qkv_one(g, q_sb, wq_t, bq_t, warm=warm)
                qkv_one(g, k_sb, wk_t, bk_t, warm=warm)
                qkv_one(g, vt_sb, wv_t, None, warm=warm)

            def rope_chunk(g):
                cols = slice(g * 512, (g + 1) * 512)
                tcols = slice((g % NBLK) * 512, (g % NBLK + 1) * 512)
                for src_t, dst_t in ((q_sb, qr), (k_sb, kr)):
                    sw = qswp.tile([128, 512], BF16, tag="sw")
                    nc.vector.stream_shuffle(
                        sw[:, :], src_t[:, cols], SWAP_MASK
                    )
                    nc.vector.tensor_mul(
                        dst_t[:, cols], src_t[:, cols], cs_t[:, tcols]
                    )
                    nc.vector.tensor_mul(
                        sw[:, :], sw[:, :],
                        cs_t[:, T + tcols.start : T + tcols.stop],
                    )
                    nc.vector.tensor_add(dst_t[:, cols], dst_t[:, cols], sw[:, :])

            def v_tiles4(g):
                # transpose v^T[:, g*512:(g+1)*512] into one [t,d] vaug group
                # tile (4 tk-tiles x [64+1 | 64+1] layout) via PE transposes
                # batched through one PSUM slot, one merged copy, one memset.
                ps = psmix.tile([128, 512], BF16, tag="mix")
                for j in range(4):
                    tt = 4 * g + j
                    nc.tensor.transpose(
                        ps[:, j * 128 : (j + 1) * 128],
                        vt_sb[:, tt * 128 : (tt + 1) * 128],
                        eye_t[:, :],
                    )
                vg = vaugp.tile([128, 4 * 130], BF16, tag="vaug")
                v4 = vg[:, :].rearrange("p (t b c) -> p t b c", t=4, b=2)
                nc.vector.memset(v4[:, :, :, 64:65], 1.0)
                nc.vector.tensor_copy(
                    v4[:, :, :, 0:64],
                    ps[:, :].rearrange("p (t b c) -> p t b c", t=4, b=2),
                )
                vaugs[g] = vg

            def attn_block(b, blk, pending_pe, feeders=()):
                # Software-pipelined emission: Tile engine queues run in
                # static program order, so PV(j) directly after S(j) would
                # stall the PE on exp(j) every iteration. Emit PV lagging S
                # by LAG iterations, and splice the previous block's R/proj
                # matmuls (pending_pe) into this block's stream so their
                # ACT/DVE dependencies are long satisfied when the PE
                # reaches them.
                LAG = 5
                gb = b * T
                base = gb + blk * 512
                ktiles = 4 * (blk + 1)
                feeders = list(feeders)
                feed_at = {2, max(3, ktiles // 2), ktiles - 1}
                yt0 = psy0.tile([128, 512], F32, tag="yt0")
                yt1 = psy1.tile([128, 512], F32, tag="yt1")
                cur_yt[0], cur_yt[1] = yt0, yt1
                stage = []  # (tk, c0, A)

                def emit_pv(tk, c0, A):
                    vg = vaugs[b * NBLK + tk // 4]
                    vo = (tk % 4) * 130
                    nc.tensor.matmul(
                        yt0[0:65, c0:512], vg[:, vo : vo + 65], A[:, c0:512],
                        start=(tk == 0), stop=(tk == ktiles - 1),
                    )
                    nc.tensor.matmul(
                        yt1[0:65, c0:512], vg[:, vo + 65 : vo + 130],
                        A[:, 512 + c0 : 1024],
                        start=(tk == 0), stop=(tk == ktiles - 1),
                    )

                for tk in range(ktiles):
                    diag = tk >= blk * 4
                    c0 = (tk - blk * 4) * 128 if diag else 0
                    S = pss.tile([128, 1024], F32, tag="spair")
                    A = apool.tile([128, 1024], BF16, tag="apair")
                    kcol = slice(gb + tk * 128, gb + (tk + 1) * 128)
                    qcol = slice(base + c0, base + 512)
                    nc.tensor.matmul(
                        S[:, c0:512], kr[0:64, kcol], qr[0:64, qcol],
                        start=True, stop=True,
                    )
                    nc.tensor.matmul(
                        S[:, 512 + c0 : 1024], kr[64:128, kcol],
                        qr[64:128, qcol], start=True, stop=True,
                    )
                    if diag:
                        s3 = S[:, :].rearrange("p (h c) -> p h c", h=2)[
                            :, :, c0:512
                        ]
                        a3 = A[:, :].rearrange("p (h c) -> p h c", h=2)[
                            :, :, c0:512
                        ]
                        nc.scalar.activation(a3, s3, Exp, scale=SCALE)
                        nc.vector.tensor_mul(
                            A[:, :].rearrange("p (h c) -> p h c", h=2)[
                                :, :, c0 : c0 + 128
                            ],
                            A[:, :].rearrange("p (h c) -> p h c", h=2)[
                                :, :, c0 : c0 + 128
                            ],
                            tri_t[:, :].rearrange("p (h c) -> p h c", h=2),
                        )
                    else:
                        nc.scalar.activation(A[:, :], S[:, :], Exp, scale=SCALE)
                    stage.append((tk, c0, A))
                    if len(stage) > LAG:
                        emit_pv(*stage.pop(0))
                    if pending_pe and tk in (1, 3, 5, 7):
                        pending_pe.pop(0)()
                    if feeders and tk in feed_at:
                        feeders.pop(0)()
                while feeders:
                    feeders.pop(0)()
                while stage:
                    emit_pv(*stage.pop(0))
                while pending_pe:
                    pending_pe.pop(0)()

                # Finalize. Take ln of the two denominator rows ([1,512],
                # fp16 — abs err < 0.004 on ln so < 0.4% on 1/den), then in
                # the deferred tail broadcast them onto the partition blocks
                # with two K=1 matmuls, turn them into 1/den with one wide
                # Exp(scale=-1), and multiply the numerators straight out of
                # the yT PSUM banks.
                lf0 = rsp.tile([1, 512], F16, tag="lf0")
                lf1 = rsp.tile([1, 512], F16, tag="lf1")
                nc.scalar.activation(lf0[:, :], yt0[64:65, 0:512], Ln)
                nc.scalar.activation(lf1[:, :], yt1[64:65, 0:512], Ln)
                ri = rip.tile([128, 512], BF16, tag="ri")

                # The tail chain Rp(PE) -> 1/den(ACT) -> yb(DVE) -> proj(PE)
                # crosses engines twice; emitted contiguously it stalls the
                # PE ~1.7us per block. Split it across four splice points in
                # the next block's stream so each hop's latency hides behind
                # attention work.
                def t1_bcast():
                    Rp = psmix.tile([128, 512], F32, tag="mix")
                    nc.tensor.matmul(
                        Rp[:, :], e2a_t[0:1, :], lf0[0:1, :],
                        start=True, stop=False,
                    )
                    nc.tensor.matmul(
                        Rp[:, :], e2b_t[0:1, :], lf1[0:1, :],
                        start=False, stop=True,
                    )
                    nc.scalar.activation(ri[:, :], Rp[:, :], Exp, scale=-1.0)
                    dummy_yt(2)

                def t2_norm():
                    nc.vector.tensor_mul(
                        yb[0:64, base : base + 512], yt0[0:64, 0:512],
                        ri[0:64, :],
                    )
                    nc.vector.tensor_mul(
                        yb[64:128, base : base + 512], yt1[0:64, 0:512],
                        ri[64:128, :],
                    )
                    dummy_yt(2)

                def t3_proj():
                    proj_block(b, blk, range(0, 4))

                def t4_proj():
                    proj_block(b, blk, range(4, C // 128))

                return [t1_bcast, t2_norm, t3_proj, t4_proj]

            def proj_block(b, blk, ccs, spread=False):
                base = b * T + blk * 512
                for cc in ccs:
                    op = psmix.tile([128, 512], F32, tag="mix")
                    nc.tensor.matmul(
                        op[:, :],
                        wo_t[:, cc * 128 : (cc + 1) * 128],
                        yb[:, base : base + 512],
                        start=True, stop=True,
                    )
                    ob = osbp.tile([128, 512], BF16, tag="ob")
                    if spread and cc % 2 == 1:
                        # drain phase: split the PSUM->SBUF casts between
                        # DVE and the (by now idle) scalar engine so the
                        # per-cc chain isn't serialized on one engine.
                        nc.scalar.activation(
                            ob[:, :], op[:, :],
                            mybir.ActivationFunctionType.Copy,
                        )
                    else:
                        nc.vector.tensor_copy(ob[:, :], op[:, :])
                    if spread:
                        eng = (nc.sync, nc.scalar, nc.gpsimd)[cc % 3]
                    else:
                        eng = (nc.sync, nc.scalar)[cc % 2]
                    eng.dma_start(
                        out=outd[cc * 128 : (cc + 1) * 128, base : base + 512],
                        in_=ob[:, :],
                    )

            # Warmup burst, then group 0's QKV with dummy matmuls spliced
            # between the k-steps (DMA-arrival jitter cover). Every later
            # group's QKV runs as feeders inside the previous block's
            # attention stream; each block's normalization and projection
            # matmuls are deferred into the next block's stream (pending_pe).
            dummy_mms(16)
            qkv_chunk(0, warm=1)
            rope_chunk(0)
            v_tiles4(0)
            pending = []
            for g in range(B * NBLK):
                h = g + 1
                if h < B * NBLK:
                    feeders = [
                        lambda h=h: qkv_one(h, q_sb, wq_t, bq_t),
                        lambda h=h: (
                            qkv_one(h, k_sb, wk_t, bk_t), rope_chunk(h)
                        ),
                        lambda h=h: (
                            qkv_one(h, vt_sb, wv_t, None), v_tiles4(h)
                        ),
                    ]
                else:
                    feeders = []
                pending = attn_block(g // NBLK, g % NBLK, pending, feeders)

            # last block's tail: run the deferred normalize/proj now, with
            # the final output DMAs spread across all three queues and the
            # casts split across DVE/ACT so the drain is short; dummy
            # matmuls keep the PE clock warm through it.
            pending[0]()
            dummy_yt(3)
            pending[1]()
            dummy_yt(3)
            proj_block(B - 1, NBLK - 1, range(0, C // 128), spread=True)

    _split_excess_waits(nc, limit=1)
    return nc


_NC_CACHE = None


def _get_nc() -> bass.Bass:
    global _NC_CACHE
    if _NC_CACHE is None:
        _NC_CACHE = _build_nc()
    return _NC_CACHE


def _prep_in_maps(x, w_attn, b_attn, w_proj, b_proj):
    bf = ml_dtypes.bfloat16
    x = np.asarray(x, np.float32)
    w_attn = np.asarray(w_attn, np.float32)
    b_attn = np.asarray(b_attn, np.float32)
    w_proj = np.asarray(w_proj, np.float32)
    b_proj = np.asarray(b_proj, np.float32)

    # blocked layout: xTb[p, ((g*NK)+k)*512 + c] = xT[k*128+p, g*512+c] so
    # every on-device column-group load is one contiguous 2D DMA slice.
    xTf = x.reshape(TT, C).T  # [C, TT]
    xTb = np.ascontiguousarray(
        xTf.reshape(NK, 128, NK, 512).transpose(1, 2, 0, 3).reshape(
            128, NK * TT
        )
    ).astype(bf)

    # rope channel permutation: pair (i, i+32) -> adjacent partitions
    # (2i, 2i+1) within each 64-wide head block. Scores are invariant since
    # q and k are permuted identically; the half-rotation becomes a swap of
    # adjacent partitions (DVE stream_shuffle).
    pi_half = np.empty(64, dtype=np.int64)
    pi_half[0::2] = np.arange(32)
    pi_half[1::2] = np.arange(32, 64)
    PI = np.concatenate([pi_half, 64 + pi_half])

    freqs = 1.0 / ROPE_THETA ** (np.arange(0, D, 2, dtype=np.float64) / D)
    ang = np.arange(T, dtype=np.float64)[:, None] * freqs[None, :]  # [T, 32]
    cosb = np.cos(ang).T  # [32, T]
    sinb = np.sin(ang).T
    cos64 = np.empty((64, T))
    sin64 = np.empty((64, T))
    cos64[0::2] = cosb
    cos64[1::2] = cosb
    sin64[0::2] = -sinb
    sin64[1::2] = sinb
    cosT = np.concatenate([cos64, cos64], axis=0)
    sinT = np.concatenate([sin64, sin64], axis=0)
    cs = np.ascontiguousarray(np.concatenate([cosT, sinT], axis=1)).astype(bf)

    r = np.arange(128)
    tri1 = (r[:, None] <= r[None, :]).astype(np.float64)
    eye1 = np.eye(128, dtype=np.float64)
    eyetri = np.ascontiguousarray(
        np.concatenate([eye1, tri1, tri1], axis=1)
    ).astype(bf)

    def karr(w):  # [C, 128] -> [128, C] with [p, k*128+j] = w[k*128+p, j]
        return np.ascontiguousarray(
            w.reshape(NK, 128, 128).transpose(1, 0, 2).reshape(128, C)
        ).astype(bf)

    maps = []
    for c in range(N_CORES):
        sl = slice(c * HD, (c + 1) * HD)
        maps.append(
            dict(
                xT=xTb,
                wq=karr(w_attn[:, 0 * C : 1 * C][:, sl][:, PI]),
                wk=karr(w_attn[:, 1 * C : 2 * C][:, sl][:, PI]),
                wv=karr(w_attn[:, 2 * C : 3 * C][:, sl]),
                wo=np.ascontiguousarray(w_proj[sl, :]).astype(bf),
                bqk=np.ascontiguousarray(
                    np.stack(
                        [
                            b_attn[0 * C : 1 * C][sl][PI],
                            b_attn[1 * C : 2 * C][sl][PI],
                        ],
                        axis=1,
                    )
                ).astype(np.float32),
                cs=cs,
                eyetri=eyetri,
            )
        )
    return maps


def _gather(results, b_eff) -> np.ndarray:
    outT = np.sum(
        np.stack([np.asarray(r["out"], np.float32) for r in results]),
        axis=0,
        dtype=np.float64,
    )
    y = outT.reshape(C, B, T).transpose(1, 2, 0) + b_eff[None, None, :]
    return np.ascontiguousarray(y).astype(np.float32)


def _bias_eff(b_attn, w_proj, b_proj):
    # v's bias is dropped on-device: softmax rows sum to 1, so its effect on
    # the output is the constant b_v @ w_proj — fold it, with c_proj's own
    # bias, into one vector added after the row-parallel partials are summed
    # (the Megatron bias-after-all-reduce placement).
    b_attn = np.asarray(b_attn, np.float64)
    return (
        np.asarray(b_proj, np.float64)
        + b_attn[2 * C : 3 * C] @ np.asarray(w_proj, np.float64)
    ).astype(np.float64)


def kernel(x, w_attn, b_attn, w_proj, b_proj, last_k_no_attend=0, window_size=0):
    from concourse.bass_utils import run_bass_kernel_spmd

    nc = _get_nc()
    maps = _prep_in_maps(x, w_attn, b_attn, w_proj, b_proj)
    res = run_bass_kernel_spmd(nc, maps, list(range(N_CORES)))
    return _gather(res.results, _bias_eff(b_attn, w_proj, b_proj))
